# revision 17
# baseline (speedup 1.0000x reference)
"""Trainium2 Bass kernel for nn_AlignGroup (hypergraph GNN message passing).

Algorithm (algebraically equivalent to the reference):
  Only i_emb = item_w + F_i @ (msg_0 + msg_1 + msg_2) is needed for the
  outputs, where msg_l are the [G, D] hyperedge messages.  Layers 1 and 2
  are collapsed through the [G, G] operators M_u = Hu @ F_u, M_i = Hi @ F_i:
      msg_0 = (Hu @ u0) @ A1_0 + (Hi @ it0) @ A2_0 + b_0
      msg_l = M_u @ (msg_{l-1} @ A1_l) + M_i @ (msg_{l-1} @ A2_l) + b_l
  so every big matrix is streamed from HBM exactly once (memory roofline)
  instead of once per layer.

Sharding (8 cores): contraction-dim sharding of the node axis (U and I each
split 8 ways).  Each core holds column-shards of Hu/Hi (pre-transposed on
host), row-shards of F, and computes partial M_u^T/M_i^T (kept core-local in
SBUF) plus partial raw messages.  Only [64, G] message partials are
all-reduced (3x 256KB).  The final item embeddings are computed per-shard,
all-gathered, and the attention/BPR head runs data-parallel over the batch;
every core ends with the full outputs (host just reads core 0).

Matmuls run in bf16 (fp32 inputs cast in-flight by SWDGE DMA); everything
else fp32.  Host-verified end-to-end absmax relative error ~1.5e-4.
"""

import numpy as np

import concourse.bass as bass
import concourse.mybir as mybir
import concourse.tile as tile
from concourse import bacc
from concourse import bass_utils
from concourse.masks import make_identity

F32 = mybir.dt.float32
F32R = mybir.dt.float32r
BF16 = mybir.dt.bfloat16
I32 = mybir.dt.int32


class Cfg:
    """Problem/tiling configuration. Defaults = the real problem."""

    def __init__(self, U=20000, I=40000, G=1000, D=64, L=3, B=2048, H=50,
                 NC=8, PT=125, SUP=5, EC=1000):
        self.U, self.I, self.G, self.D, self.L, self.B, self.H = U, I, G, D, L, B, H
        self.NC = NC          # number of cores
        self.PT = PT          # node/group tile rows (<= 128)
        self.UC, self.IC, self.BC = U // NC, I // NC, B // NC
        self.UK, self.IK = self.UC // PT, self.IC // PT   # node k-tiles per side
        self.GK = G // PT                                  # group tiles
        self.SUP = SUP        # k-tiles per streaming super-tile
        self.EC = EC          # emb column chunk (== psum tile width)
        assert self.UC % PT == 0 and self.IC % PT == 0 and self.G % PT == 0
        assert self.IC % EC == 0
        assert G * 4 <= 4096  # [*, G] fp32 psum tile must fit 2 banks


def _nsplit(n, cap=512):
    """Split width n into matmul-legal (<=512) column chunks."""
    out, off = [], 0
    while off < n:
        w = min(cap, n - off)
        out.append((off, w))
        off += w
    return out


def build_nc(cfg: Cfg, debug: bool = False, stop_phase: int = 99) -> bacc.Bacc:
    nc = bacc.Bacc("TRN2", target_bir_lowering=False, debug=debug,
                   num_devices=cfg.NC)

    U, I, G, D, L, B, H = cfg.U, cfg.I, cfg.G, cfg.D, cfg.L, cfg.B, cfg.H
    UC, IC, BC = cfg.UC, cfg.IC, cfg.BC

    io = {}
    def din(name, shape, dtype=F32):
        io[name] = nc.dram_tensor(name, shape, dtype, kind="ExternalInput").ap()
    din("hu_t", [UC, G], F32R); din("hi_t", [IC, G], F32R)
    din("fu", [UC, G], F32R); din("fi", [IC, G], F32R)
    din("fi_t", [G, IC], F32R)
    din("u0", [UC, D], F32R); din("it0", [IC, D], F32R)
    din("proj_w", [L, 2, D, D]); din("proj_b", [L, D]); din("attn_wt", [128, D])
    din("hist_idx", [BC, H], I32); din("pos_idx", [BC, 1], I32)
    din("neg_idx", [BC, 1], I32); din("mask_i", [BC, H], I32)
    io["loss_out"] = nc.dram_tensor("loss_out", [1], F32, kind="ExternalOutput").ap()
    io["pos_out"] = nc.dram_tensor("pos_out", [B], F32, kind="ExternalOutput").ap()

    with tile.TileContext(nc) as tc:
        _emit(tc, cfg, io, stop_phase)
    nc.compile()
    return nc


def _emit(tc, cfg, io, stop_phase=99):
    nc = tc.nc
    U, I, G, D, L, B, H = cfg.U, cfg.I, cfg.G, cfg.D, cfg.L, cfg.B, cfg.H
    NC, PT, UC, IC, BC = cfg.NC, cfg.PT, cfg.UC, cfg.IC, cfg.BC
    UK, IK, GK, SUP, EC = cfg.UK, cfg.IK, cfg.GK, cfg.SUP, cfg.EC
    rg = [list(range(NC))]
    gsplit = _nsplit(G)

    # ---------------- pools ----------------
    const = tc.alloc_tile_pool(name="const", bufs=1)
    streams = tc.alloc_tile_pool(name="streams", bufs=7)       # f32r [PT, G]
    macc_p = tc.alloc_tile_pool(name="macc", bufs=1)           # M^T accumulators
    small = tc.alloc_tile_pool(name="small", bufs=2)           # [64, G] working
    natp = tc.alloc_tile_pool(name="natp", bufs=1)             # persistent naturals
    gath = tc.alloc_tile_pool(name="gath", bufs=2)             # gather-phase tiles
    psum = tc.alloc_tile_pool(name="psum", bufs=4, space="PSUM")
    dram = tc.alloc_tile_pool(name="dram", bufs=1, space="DRAM")

    # ---------------- constants ----------------
    ident_b = const.tile([D, D], BF16, tag="ident_b")
    make_identity(nc, ident_b)
    ident_f = const.tile([D, D], F32, tag="ident_f")
    make_identity(nc, ident_f)

    proj_sb, projb_sb = [], []
    for l in range(L):
        row = []
        for s in range(2):
            t = const.tile([D, D], BF16, tag=f"proj_{l}_{s}")
            nc.gpsimd.dma_start(out=t, in_=io["proj_w"][l, s])
            row.append(t)
        proj_sb.append(row)
        bt = const.tile([D, 1], F32, tag=f"projb_{l}")
        nc.sync.dma_start(out=bt, in_=io["proj_b"][l, :])
        projb_sb.append(bt)

    attn_sb = const.tile([128, D], F32, tag="attn")
    nc.sync.dma_start(out=attn_sb, in_=io["attn_wt"])

    # layer-0 node features, packed [PT, K*D] (k-major in free dim)
    u0b = const.tile([PT, UK * D], F32R, tag="u0b")
    nc.sync.dma_start(
        out=u0b[:].rearrange("p (k d) -> p k d", k=UK),
        in_=io["u0"].rearrange("(k p) d -> p k d", p=PT))
    it0b = const.tile([PT, IK * D], F32R, tag="it0b")
    nc.sync.dma_start(
        out=it0b[:].rearrange("p (k d) -> p k d", k=IK),
        in_=io["it0"].rearrange("(k p) d -> p k d", p=PT))
    it0f = it0b

    ones8 = const.tile([NC, 1], F32, tag="ones8")
    nc.vector.memset(ones8, 1.0)

    def _early_out():
        zt = gath.tile([NC, BC], F32, tag="pall", name="zero_out")
        nc.vector.memset(zt, 0.0)
        nc.sync.dma_start(
            out=io["pos_out"].rearrange("(n b) -> n b", b=BC), in_=zt)
        z1 = gath.tile([1, 1], F32, tag="lsb", name="zero_loss")
        nc.vector.memset(z1, 0.0)
        nc.sync.dma_start(out=io["loss_out"][0:1], in_=z1)
        for p in (dram, psum, gath, natp, small, macc_p, streams, const):
            p.release()

    # ---------------- pass 1: stream Hu^T/Fu then Hi^T/Fi ----------------
    macc, raw_ps = {}, {}
    sides = (("u", io["hu_t"], io["fu"], UK, u0b),
             ("i", io["hi_t"], io["fi"], IK, it0b))
    for side, h_in, f_in, KT, w0sb in sides:
        macc[side] = [macc_p.tile([PT, G], BF16, tag=f"macc_{side}_{m}",
                                  name=f"macc_{side}_{m}")
                      for m in range(GK)]
        raw_ps[side] = psum.tile([D, G], F32, tag="big", name=f"raw_{side}")
        n_sup = (KT + SUP - 1) // SUP
        for s in range(n_sup):
            ks = list(range(s * SUP, min((s + 1) * SUP, KT)))
            hT, fT = {}, {}
            for k in ks:
                hT[k] = streams.tile([PT, G], F32R, tag="hstream",
                                     name=f"h_{side}_{k}")
                nc.sync.dma_start(out=hT[k], in_=h_in[k * PT:(k + 1) * PT, :])
                fT[k] = streams.tile([PT, G], F32R, tag="fstream",
                                     name=f"f_{side}_{k}")
                nc.sync.dma_start(out=fT[k], in_=f_in[k * PT:(k + 1) * PT, :])
            # raw0 partial: [D, G] += w0[k].T @ hT[k]
            for k in ks:
                for (off, w) in gsplit:
                    nc.tensor.matmul(
                        raw_ps[side][:, off:off + w],
                        lhsT=w0sb[:, k * D:(k + 1) * D],
                        rhs=hT[k][:, off:off + w],
                        start=(k == 0), stop=(k == KT - 1))
            # M^T partial: [G(m), G] += f[k][:, m].T @ hT[k]
            for m in range(GK):
                ps = psum.tile([PT, G], F32, tag="big", name=f"mps_{side}_{s}_{m}")
                for j, k in enumerate(ks):
                    for (off, w) in gsplit:
                        nc.tensor.matmul(
                            ps[:, off:off + w],
                            lhsT=fT[k][:, m * PT:(m + 1) * PT],
                            rhs=hT[k][:, off:off + w],
                            start=(j == 0), stop=(j == len(ks) - 1))
                if s == 0:
                    nc.vector.tensor_copy(macc[side][m], ps)
                else:
                    nc.vector.tensor_add(macc[side][m], macc[side][m], ps)

    if stop_phase <= 1:
        _early_out()
        return

    # ---------------- messages ----------------
    def allreduce(src_sb, tag):
        cin = dram.tile([D, G], F32, tag="cc_in", bufs=2, name=f"ccin_{tag}")
        cout = dram.tile([D, G], F32, tag="cc_out", bufs=2, name=f"ccout_{tag}",
                         addr_space="Shared")
        nc.sync.dma_start(out=cin, in_=src_sb)
        nc.gpsimd.collective_compute(
            "AllReduce", mybir.AluOpType.add,
            ins=[cin.opt()], outs=[cout.opt()], replica_groups=rg)
        dst = small.tile([D, G], F32, tag="msgT", bufs=6, name=f"msgT_{tag}")
        nc.sync.dma_start(out=dst, in_=cout)
        return dst

    rawb = {}
    for side in ("u", "i"):
        rawb[side] = small.tile([D, G], BF16, tag=f"rawb_{side}",
                                name=f"rawb_{side}")
        nc.vector.tensor_copy(rawb[side], raw_ps[side])
    mps = psum.tile([D, G], F32, tag="big", name="proj0")
    for (off, w) in gsplit:
        nc.tensor.matmul(mps[:, off:off + w], lhsT=proj_sb[0][0],
                         rhs=rawb["u"][:, off:off + w], start=True, stop=False)
        nc.tensor.matmul(mps[:, off:off + w], lhsT=proj_sb[0][1],
                         rhs=rawb["i"][:, off:off + w], start=False, stop=True)
    m0p = small.tile([D, G], F32, tag="msgT", bufs=6, name="msg0_part")
    nc.vector.tensor_copy(m0p, mps)
    msgT = [None] * L
    msgT[0] = allreduce(m0p, "l0")
    nc.vector.tensor_scalar(out=msgT[0], in0=msgT[0], scalar1=projb_sb[0],
                            scalar2=None, op0=mybir.AluOpType.add)

    if stop_phase <= 2:
        _early_out()
        return

    def transpose_to_nat(srcT_b, tag):
        """[D, G] bf16 -> GK natural tiles [PT, D] bf16 (PE transpose)."""
        nats = []
        for k in range(GK):
            pst = psum.tile([PT, D], BF16, tag="big", name=f"tp_{tag}_{k}")
            nc.tensor.transpose(pst, srcT_b[:, k * PT:(k + 1) * PT], ident_b)
            nt = natp.tile([PT, D], BF16, tag=f"nat_{tag}_{k}",
                           name=f"nat_{tag}_{k}")
            nc.vector.tensor_copy(nt, pst)
            nats.append(nt)
        return nats

    for l in range(1, L):
        mb = small.tile([D, G], BF16, tag="msgTb", name=f"msgTb_{l}")
        nc.vector.tensor_copy(mb, msgT[l - 1])
        zps = psum.tile([D, G], F32, tag="big", name=f"z_{l}")
        for si, side in enumerate(("u", "i")):
            qps = psum.tile([D, G], F32, tag="big", name=f"q_{l}_{side}")
            for (off, w) in gsplit:
                nc.tensor.matmul(qps[:, off:off + w], lhsT=proj_sb[l][si],
                                 rhs=mb[:, off:off + w], start=True, stop=True)
            qb = small.tile([D, G], BF16, tag="qTb", name=f"qTb_{l}_{side}")
            nc.vector.tensor_copy(qb, qps)
            qn = transpose_to_nat(qb, f"q{l}{side}")
            for k in range(GK):
                for (off, w) in gsplit:
                    nc.tensor.matmul(
                        zps[:, off:off + w], lhsT=qn[k],
                        rhs=macc[side][k][:, off:off + w],
                        start=(si == 0 and k == 0),
                        stop=(si == 1 and k == GK - 1))
        zp = small.tile([D, G], F32, tag="msgT", bufs=6, name=f"z_part_{l}")
        nc.vector.tensor_copy(zp, zps)
        msgT[l] = allreduce(zp, f"l{l}")
        nc.vector.tensor_scalar(out=msgT[l], in0=msgT[l], scalar1=projb_sb[l],
                                scalar2=None, op0=mybir.AluOpType.add)

    msum = small.tile([D, G], F32, tag="msgT", bufs=6, name="msg_sum_T")
    nc.vector.tensor_add(msum, msgT[0], msgT[1])
    for l in range(2, L):
        nc.vector.tensor_add(msum, msum, msgT[l])
    msn = []
    for k in range(GK):
        pst = psum.tile([PT, D], F32, tag="big", name=f"tp_ms_{k}")
        nc.tensor.transpose(pst, msum[:, k * PT:(k + 1) * PT], ident_f)
        nt = natp.tile([PT, D], F32R, tag=f"nat_ms_{k}", name=f"nat_ms_{k}")
        nc.vector.tensor_copy(nt, pst)
        msn.append(nt)

    if stop_phase <= 3:
        _early_out()
        return

    # ---------------- final item embeddings ----------------
    iemb_b = dram.tile([IC, D], F32, tag="iemb_local")
    iemb_all = dram.tile([NC * IC, D], F32, tag="iemb_all",
                         addr_space="Shared")
    ecsplit = _nsplit(EC)
    for c in range(IC // EC):
        eps = psum.tile([D, EC], F32, tag="big", name=f"embT_{c}")
        for k in range(GK):
            ft = streams.tile([PT, EC], F32R, tag="fstream", name=f"fiT_{c}_{k}")
            nc.sync.dma_start(
                out=ft, in_=io["fi_t"][k * PT:(k + 1) * PT, c * EC:(c + 1) * EC])
            for (off, w) in ecsplit:
                nc.tensor.matmul(eps[:, off:off + w], lhsT=msn[k],
                                 rhs=ft[:, off:off + w],
                                 start=(k == 0), stop=(k == GK - 1))
        ets = small.tile([D, EC], F32, tag="embT_sb", name=f"embT_sb_{c}")
        nc.vector.tensor_copy(ets, eps)
        for t in range(EC // PT):
            kg = c * (EC // PT) + t
            pst = psum.tile([PT, D], F32, tag="big", name=f"etp_{c}_{t}")
            nc.tensor.transpose(pst, ets[:, t * PT:(t + 1) * PT], ident_f)
            ie = gath.tile([PT, D], F32, tag="iemb_t")
            nc.vector.tensor_add(ie, pst, it0f[:, kg * D:(kg + 1) * D])
            nc.sync.dma_start(out=iemb_b[kg * PT:(kg + 1) * PT, :], in_=ie)

    nc.gpsimd.collective_compute(
        "AllGather", mybir.AluOpType.bypass,
        ins=[iemb_b.opt()], outs=[iemb_all.opt()], replica_groups=rg)

    if stop_phase <= 4:
        _early_out()
        return

    # ---------------- attention / BPR head (batch shard) ----------------
    pred_in = dram.tile([2, BC], F32, tag="pred_in")
    pred_ag = dram.tile([2 * NC, BC], F32, tag="pred_ag",
                        addr_space="Shared")

    boff = 0
    while boff < BC:
        bp = min(128, BC - boff)
        idx = gath.tile([bp, H], I32, tag="idx")
        nc.sync.dma_start(out=idx, in_=io["hist_idx"][boff:boff + bp, :])
        pidx = gath.tile([bp, 1], I32, tag="pidx")
        nc.sync.dma_start(out=pidx, in_=io["pos_idx"][boff:boff + bp, :])
        nidx = gath.tile([bp, 1], I32, tag="nidx")
        nc.sync.dma_start(out=nidx, in_=io["neg_idx"][boff:boff + bp, :])
        mski = gath.tile([bp, H], I32, tag="mski")
        nc.sync.dma_start(out=mski, in_=io["mask_i"][boff:boff + bp, :])

        hist = gath.tile([bp, H * D], F32, tag="hist")
        # HW indirect DMA consumes ONE index per partition -> one gather per h
        for h in range(H):
            nc.gpsimd.indirect_dma_start(
                out=hist[:, h * D:(h + 1) * D], out_offset=None, in_=iemb_all,
                in_offset=bass.IndirectOffsetOnAxis(ap=idx[:, h:h + 1], axis=0))
        ipos = gath.tile([bp, D], F32, tag="ipos")
        nc.gpsimd.indirect_dma_start(
            out=ipos, out_offset=None, in_=iemb_all,
            in_offset=bass.IndirectOffsetOnAxis(ap=pidx[:, :], axis=0))
        ineg = gath.tile([bp, D], F32, tag="ineg")
        nc.gpsimd.indirect_dma_start(
            out=ineg, out_offset=None, in_=iemb_all,
            in_offset=bass.IndirectOffsetOnAxis(ap=nidx[:, :], axis=0))

        maskf = gath.tile([bp, H], F32, tag="maskf")
        nc.vector.tensor_copy(maskf, mski)

        # logits[b, h] = sum_d hist[b, h, d] * attn_w[d]
        # (tensor_tensor_reduce is broken on this HW path -> NRT 101; use
        # scalar_tensor_tensor with accum_out instead)
        logits = gath.tile([bp, H], F32, tag="logits")
        lsc = gath.tile([bp, D], F32, tag="lsc")
        for h in range(H):
            nc.vector.scalar_tensor_tensor(
                out=lsc, in0=hist[:, h * D:(h + 1) * D], scalar=1.0,
                in1=attn_sb[:bp, :], op0=mybir.AluOpType.mult,
                op1=mybir.AluOpType.mult, accum_out=logits[:, h:h + 1])

        # masked softmax over H (mask entries are exactly 0/1)
        neg_big = gath.tile([bp, H], F32, tag="neg_big")
        nc.vector.tensor_scalar(out=neg_big, in0=maskf, scalar1=1.0,
                                scalar2=60.0, op0=mybir.AluOpType.subtract,
                                op1=mybir.AluOpType.mult)
        ml = gath.tile([bp, H], F32, tag="ml")
        nc.vector.tensor_mul(ml, logits, maskf)
        nc.vector.tensor_add(ml, ml, neg_big)
        rmax = gath.tile([bp, 1], F32, tag="rmax")
        nc.vector.reduce_max(out=rmax, in_=ml, axis=mybir.AxisListType.X)
        nc.vector.tensor_scalar(out=ml, in0=ml, scalar1=rmax, scalar2=None,
                                op0=mybir.AluOpType.subtract)
        ex = gath.tile([bp, H], F32, tag="ex")
        nc.scalar.activation(ex, ml, mybir.ActivationFunctionType.Exp)
        nc.vector.tensor_mul(ex, ex, maskf)
        ssum = gath.tile([bp, 1], F32, tag="ssum")
        nc.vector.reduce_sum(out=ssum, in_=ex, axis=mybir.AxisListType.X)
        rinv = gath.tile([bp, 1], F32, tag="rinv")
        nc.vector.reciprocal(rinv, ssum)
        nc.vector.tensor_scalar(out=ex, in0=ex, scalar1=rinv, scalar2=None,
                                op0=mybir.AluOpType.mult)

        # g = sum_h attn[b, h] * hist[b, h, :]
        g = gath.tile([bp, D], F32, tag="g")
        nc.vector.tensor_scalar(out=g, in0=hist[:, 0:D], scalar1=ex[:, 0:1],
                                scalar2=None, op0=mybir.AluOpType.mult)
        for h in range(1, H):
            nc.vector.scalar_tensor_tensor(
                out=g, in0=hist[:, h * D:(h + 1) * D], scalar=ex[:, h:h + 1],
                in1=g, op0=mybir.AluOpType.mult, op1=mybir.AluOpType.add)

        pp = gath.tile([bp, D], F32, tag="pp")
        posp = gath.tile([bp, 1], F32, tag="posp")
        nc.vector.tensor_mul(pp, g, ipos)
        nc.vector.reduce_sum(out=posp, in_=pp, axis=mybir.AxisListType.X)
        negp = gath.tile([bp, 1], F32, tag="negp")
        nc.vector.tensor_mul(pp, g, ineg)
        nc.vector.reduce_sum(out=negp, in_=pp, axis=mybir.AxisListType.X)

        nc.sync.dma_start(out=pred_in[0, boff:boff + bp], in_=posp)
        nc.sync.dma_start(out=pred_in[1, boff:boff + bp], in_=negp)
        boff += bp

    if stop_phase <= 5:
        _early_out()
        return

    nc.gpsimd.collective_compute(
        "AllGather", mybir.AluOpType.bypass,
        ins=[pred_in.opt()], outs=[pred_ag.opt()], replica_groups=rg)

    # ---------------- loss (computed redundantly on every core) --------------
    ag3 = pred_ag[:].rearrange("(n two) b -> n two b", two=2)
    pall = gath.tile([NC, BC], F32, tag="pall")
    nc.sync.dma_start(out=pall, in_=ag3[:, 0, :])
    nall = gath.tile([NC, BC], F32, tag="nall")
    nc.sync.dma_start(out=nall, in_=ag3[:, 1, :])
    x = gath.tile([NC, BC], F32, tag="x")
    nc.vector.tensor_sub(x, nall, pall)
    # softplus(x) for small |x| (preds are O(0.1)): even-poly Taylor series,
    # abs err < 2e-6 for |x| <= 1.  (No Softplus/Ln ACT table on TRN2.)
    y = gath.tile([NC, BC], F32, tag="y")
    nc.vector.tensor_mul(y, x, x)
    sp = gath.tile([NC, BC], F32, tag="sp")
    nc.vector.tensor_scalar(out=sp, in0=y, scalar1=1.0 / 2880.0,
                            scalar2=-1.0 / 192.0, op0=mybir.AluOpType.mult,
                            op1=mybir.AluOpType.add)
    nc.vector.tensor_mul(sp, sp, y)
    nc.vector.tensor_scalar(out=sp, in0=sp, scalar1=0.125, scalar2=None,
                            op0=mybir.AluOpType.add)
    nc.vector.tensor_mul(sp, sp, y)
    xh = gath.tile([NC, BC], F32, tag="xh")
    nc.vector.tensor_scalar(out=xh, in0=x, scalar1=0.5,
                            scalar2=float(np.log(2.0)),
                            op0=mybir.AluOpType.mult, op1=mybir.AluOpType.add)
    nc.vector.tensor_add(sp, sp, xh)
    prt = gath.tile([NC, 1], F32, tag="prt")
    nc.vector.reduce_sum(out=prt, in_=sp, axis=mybir.AxisListType.X)
    lps = psum.tile([1, 1], F32, tag="big", name="loss_ps")
    nc.tensor.matmul(lps, lhsT=prt, rhs=ones8, start=True, stop=True)
    lsb = gath.tile([1, 1], F32, tag="lsb")
    nc.scalar.activation(lsb, lps, mybir.ActivationFunctionType.Copy,
                         scale=1.0 / B)
    nc.sync.dma_start(out=io["loss_out"][0:1], in_=lsb)
    nc.sync.dma_start(
        out=io["pos_out"].rearrange("(n b) -> n b", b=BC), in_=ag3[:, 0, :])

    for p in (dram, psum, gath, natp, small, macc_p, streams, const):
        p.release()


# ---------------------------------------------------------------------------
# host side
# ---------------------------------------------------------------------------

def shard_inputs(cfg: Cfg, inputs: dict) -> list:
    U, I, G, D, L, B, H, NC = (cfg.U, cfg.I, cfg.G, cfg.D, cfg.L, cfg.B,
                               cfg.H, cfg.NC)
    UC, IC, BC = cfg.UC, cfg.IC, cfg.BC
    f32 = np.float32
    Hu = np.asarray(inputs["user_hyper"], f32)
    Hi = np.asarray(inputs["item_hyper"], f32)
    F = np.asarray(inputs["full_hyper"], f32)
    user_w = np.asarray(inputs["user_w"], f32)
    item_w = np.asarray(inputs["item_w"], f32)
    agg_w = np.asarray(inputs["agg_w"], f32)
    agg_b = np.asarray(inputs["agg_b"], f32)
    attn_w = np.asarray(inputs["attn_w"], f32)
    hist = np.asarray(inputs["group_history"]).astype(np.int32)
    mask = np.asarray(inputs["group_mask"]).astype(np.int32)
    pos = np.asarray(inputs["pos_item_inputs"]).astype(np.int32).reshape(B, 1)
    neg = np.asarray(inputs["neg_item_inputs"]).astype(np.int32).reshape(B, 1)

    proj_w = np.stack([
        np.stack([agg_w[l][:, :D].T, agg_w[l][:, D:].T]) for l in range(L)
    ]).astype(f32)
    attn_wt = np.tile(attn_w.reshape(1, D), (128, 1)).astype(f32)

    maps = []
    for k in range(NC):
        us = slice(k * UC, (k + 1) * UC)
        isl = slice(k * IC, (k + 1) * IC)
        bs = slice(k * BC, (k + 1) * BC)
        fi_k = F[U:][isl]
        maps.append({
            "hu_t": np.ascontiguousarray(Hu[:, us].T),
            "hi_t": np.ascontiguousarray(Hi[:, isl].T),
            "fu": np.ascontiguousarray(F[:U][us]),
            "fi": np.ascontiguousarray(fi_k),
            "fi_t": np.ascontiguousarray(fi_k.T),
            "u0": np.ascontiguousarray(user_w[us]),
            "it0": np.ascontiguousarray(item_w[isl]),
            "proj_w": proj_w,
            "proj_b": np.ascontiguousarray(agg_b),
            "attn_wt": attn_wt,
            "hist_idx": np.ascontiguousarray(hist[bs]),
            "pos_idx": np.ascontiguousarray(pos[bs]),
            "neg_idx": np.ascontiguousarray(neg[bs]),
            "mask_i": np.ascontiguousarray(mask[bs]),
        })
    return maps


_CACHE = {}


def get_nc(cfg: Cfg, debug=False):
    key = (tuple(sorted((k, v) for k, v in cfg.__dict__.items())), debug)
    if key not in _CACHE:
        _CACHE[key] = build_nc(cfg, debug=debug)
    return _CACHE[key]


def kernel(**inputs):
    cfg = Cfg()
    nc = get_nc(cfg)
    in_maps = shard_inputs(cfg, inputs)
    res = bass_utils.run_bass_kernel_spmd(
        nc, in_maps, core_ids=list(range(cfg.NC)))
    out = res.results[0]
    loss = np.float32(np.asarray(out["loss_out"]).reshape(())[()])
    pos_pred = np.asarray(out["pos_out"], np.float32).reshape(cfg.B)
    return loss, pos_pred


# revision 20
# speedup vs baseline: 1.0618x; 1.0618x over previous
"""Trainium2 Bass kernel for nn_AlignGroup (hypergraph GNN message passing).

Algorithm (algebraically equivalent to the reference):
  Only i_emb = item_w + F_i @ (msg_0 + msg_1 + msg_2) is needed for the
  outputs, where msg_l are the [G, D] hyperedge messages.  Layers 1 and 2
  are collapsed through the [G, G] operators M_u = Hu @ F_u, M_i = Hi @ F_i:
      msg_0 = (Hu @ u0) @ A1_0 + (Hi @ it0) @ A2_0 + b_0
      msg_l = M_u @ (msg_{l-1} @ A1_l) + M_i @ (msg_{l-1} @ A2_l) + b_l
  so every big matrix is streamed from HBM exactly once (memory roofline)
  instead of once per layer.

Sharding (8 cores): contraction-dim sharding of the node axis (U and I each
split 8 ways).  Each core holds column-shards of Hu/Hi (pre-transposed on
host), row-shards of F, and computes partial M_u^T/M_i^T (kept core-local in
SBUF) plus partial raw messages.  Only [64, G] message partials are
all-reduced (3x 256KB).  The final item embeddings are computed per-shard,
all-gathered, and the attention/BPR head runs data-parallel over the batch;
every core ends with the full outputs (host just reads core 0).

Matmuls run in bf16 (fp32 inputs cast in-flight by SWDGE DMA); everything
else fp32.  Host-verified end-to-end absmax relative error ~1.5e-4.
"""

import numpy as np

import concourse.bass as bass
import concourse.mybir as mybir
import concourse.tile as tile
from concourse import bacc
from concourse import bass_utils
from concourse.masks import make_identity

F32 = mybir.dt.float32
F32R = mybir.dt.float32r
BF16 = mybir.dt.bfloat16
I32 = mybir.dt.int32


class Cfg:
    """Problem/tiling configuration. Defaults = the real problem."""

    def __init__(self, U=20000, I=40000, G=1000, D=64, L=3, B=2048, H=50,
                 NC=8, PT=125, SUP=5, EC=1000):
        self.U, self.I, self.G, self.D, self.L, self.B, self.H = U, I, G, D, L, B, H
        self.NC = NC          # number of cores
        self.PT = PT          # node/group tile rows (<= 128)
        self.UC, self.IC, self.BC = U // NC, I // NC, B // NC
        self.UK, self.IK = self.UC // PT, self.IC // PT   # node k-tiles per side
        self.GK = G // PT                                  # group tiles
        self.SUP = SUP        # k-tiles per streaming super-tile
        self.EC = EC          # emb column chunk (== psum tile width)
        assert self.UC % PT == 0 and self.IC % PT == 0 and self.G % PT == 0
        assert self.IC % EC == 0
        assert G * 4 <= 4096  # [*, G] fp32 psum tile must fit 2 banks


def _nsplit(n, cap=512):
    """Split width n into matmul-legal (<=512) column chunks."""
    out, off = [], 0
    while off < n:
        w = min(cap, n - off)
        out.append((off, w))
        off += w
    return out


def build_nc(cfg: Cfg, debug: bool = False, stop_phase: int = 99) -> bacc.Bacc:
    nc = bacc.Bacc("TRN2", target_bir_lowering=False, debug=debug,
                   num_devices=cfg.NC)

    U, I, G, D, L, B, H = cfg.U, cfg.I, cfg.G, cfg.D, cfg.L, cfg.B, cfg.H
    UC, IC, BC = cfg.UC, cfg.IC, cfg.BC

    io = {}
    def din(name, shape, dtype=F32):
        io[name] = nc.dram_tensor(name, shape, dtype, kind="ExternalInput").ap()
    din("hu_t", [UC, G], F32R); din("hi_t", [IC, G], F32R)
    din("fu", [UC, G], F32R); din("fi", [IC, G], F32R)
    din("fi_t", [G, IC], F32R)
    din("u0", [UC, D], F32R); din("it0", [IC, D], F32R)
    din("proj_w", [L, 2, D, D]); din("proj_b", [L, D]); din("attn_wt", [128, D])
    din("hist_idx", [BC, H], I32); din("pos_idx", [BC, 1], I32)
    din("neg_idx", [BC, 1], I32); din("mask_i", [BC, H], I32)
    io["loss_out"] = nc.dram_tensor("loss_out", [1], F32, kind="ExternalOutput").ap()
    io["pos_out"] = nc.dram_tensor("pos_out", [B], F32, kind="ExternalOutput").ap()

    with tile.TileContext(nc) as tc:
        _emit(tc, cfg, io, stop_phase)
    nc.compile()
    return nc


def _emit(tc, cfg, io, stop_phase=99):
    nc = tc.nc
    U, I, G, D, L, B, H = cfg.U, cfg.I, cfg.G, cfg.D, cfg.L, cfg.B, cfg.H
    NC, PT, UC, IC, BC = cfg.NC, cfg.PT, cfg.UC, cfg.IC, cfg.BC
    UK, IK, GK, SUP, EC = cfg.UK, cfg.IK, cfg.GK, cfg.SUP, cfg.EC
    rg = [list(range(NC))]
    gsplit = _nsplit(G)

    # ---------------- pools ----------------
    # streams is allocated last (stack top) and released after the final
    # embedding phase; the gather-phase pool then reuses its SBUF region.
    const = tc.alloc_tile_pool(name="const", bufs=1)
    macc_p = tc.alloc_tile_pool(name="macc", bufs=1)           # M^T accumulators
    small = tc.alloc_tile_pool(name="small", bufs=2)           # [64, G] working
    natp = tc.alloc_tile_pool(name="natp", bufs=1)             # persistent naturals
    psum = tc.alloc_tile_pool(name="psum", bufs=4, space="PSUM")
    dram = tc.alloc_tile_pool(name="dram", bufs=1, space="DRAM")
    streams = tc.alloc_tile_pool(name="streams", bufs=10)      # f32r [PT, G]
    gath = None  # allocated after streams is released

    # ---------------- constants ----------------
    ident_b = const.tile([D, D], BF16, tag="ident_b")
    make_identity(nc, ident_b)
    ident_f = const.tile([D, D], F32, tag="ident_f")
    make_identity(nc, ident_f)

    proj_sb, projb_sb = [], []
    for l in range(L):
        row = []
        for s in range(2):
            t = const.tile([D, D], BF16, tag=f"proj_{l}_{s}")
            nc.gpsimd.dma_start(out=t, in_=io["proj_w"][l, s])
            row.append(t)
        proj_sb.append(row)
        bt = const.tile([D, 1], F32, tag=f"projb_{l}")
        nc.sync.dma_start(out=bt, in_=io["proj_b"][l, :])
        projb_sb.append(bt)

    attn_sb = const.tile([128, D], F32, tag="attn")
    nc.sync.dma_start(out=attn_sb, in_=io["attn_wt"])

    # layer-0 node features, packed [PT, K*D] (k-major in free dim)
    u0b = const.tile([PT, UK * D], F32R, tag="u0b")
    nc.sync.dma_start(
        out=u0b[:].rearrange("p (k d) -> p k d", k=UK),
        in_=io["u0"].rearrange("(k p) d -> p k d", p=PT))
    it0b = const.tile([PT, IK * D], F32R, tag="it0b")
    nc.sync.dma_start(
        out=it0b[:].rearrange("p (k d) -> p k d", k=IK),
        in_=io["it0"].rearrange("(k p) d -> p k d", p=PT))
    it0f = it0b

    ones8 = const.tile([NC, 1], F32, tag="ones8")
    nc.vector.memset(ones8, 1.0)

    # tiny throwaway AllReduce issued up front: warms the ncfw collective
    # path while pass-1 streams, so the first real AR doesn't pay cold-start
    wrm_i = dram.tile([NC, 16], F32, tag="wrm_i", name="wrm_i")
    wrm_o = dram.tile([NC, 16], F32, tag="wrm_o", name="wrm_o",
                      addr_space="Shared")
    wrm_s = const.tile([NC, 16], F32, tag="wrm_s")
    nc.vector.memset(wrm_s, 0.0)
    nc.sync.dma_start(out=wrm_i, in_=wrm_s)
    nc.gpsimd.collective_compute(
        "AllReduce", mybir.AluOpType.add,
        ins=[wrm_i.opt()], outs=[wrm_o.opt()], replica_groups=rg)

    def _early_out():
        g = gath if gath is not None else small
        zt = g.tile([NC, BC], F32, tag="pall", name="zero_out")
        nc.vector.memset(zt, 0.0)
        nc.sync.dma_start(
            out=io["pos_out"].rearrange("(n b) -> n b", b=BC), in_=zt)
        z1 = g.tile([1, 1], F32, tag="lsb", name="zero_loss")
        nc.vector.memset(z1, 0.0)
        nc.sync.dma_start(out=io["loss_out"][0:1], in_=z1)
        pools = [gath] if gath is not None else [streams]
        if gath is None:
            pools = [streams]
        else:
            pools = [gath]
        for p in pools + [dram, psum, natp, small, macc_p, const]:
            p.release()

    # ---------------- pass 1: stream Hu^T/Fu then Hi^T/Fi ----------------
    macc, raw_ps = {}, {}
    sides = (("u", io["hu_t"], io["fu"], UK, u0b),
             ("i", io["hi_t"], io["fi"], IK, it0b))
    for side, h_in, f_in, KT, w0sb in sides:
        macc[side] = [macc_p.tile([PT, G], BF16, tag=f"macc_{side}_{m}",
                                  name=f"macc_{side}_{m}")
                      for m in range(GK)]
        raw_ps[side] = psum.tile([D, G], F32, tag="big", name=f"raw_{side}")
        n_sup = (KT + SUP - 1) // SUP
        for s in range(n_sup):
            ks = list(range(s * SUP, min((s + 1) * SUP, KT)))
            hT, fT = {}, {}
            for k in ks:
                hT[k] = streams.tile([PT, G], F32R, tag="hstream",
                                     name=f"h_{side}_{k}")
                nc.sync.dma_start(out=hT[k], in_=h_in[k * PT:(k + 1) * PT, :])
                fT[k] = streams.tile([PT, G], F32R, tag="fstream",
                                     name=f"f_{side}_{k}")
                nc.sync.dma_start(out=fT[k], in_=f_in[k * PT:(k + 1) * PT, :])
            # raw0 partial: [D, G] += w0[k].T @ hT[k]
            for k in ks:
                for (off, w) in gsplit:
                    nc.tensor.matmul(
                        raw_ps[side][:, off:off + w],
                        lhsT=w0sb[:, k * D:(k + 1) * D],
                        rhs=hT[k][:, off:off + w],
                        start=(k == 0), stop=(k == KT - 1))
            # M^T partial: [G(m), G] += f[k][:, m].T @ hT[k]
            for m in range(GK):
                ps = psum.tile([PT, G], F32, tag="big", name=f"mps_{side}_{s}_{m}")
                for j, k in enumerate(ks):
                    for (off, w) in gsplit:
                        nc.tensor.matmul(
                            ps[:, off:off + w],
                            lhsT=fT[k][:, m * PT:(m + 1) * PT],
                            rhs=hT[k][:, off:off + w],
                            start=(j == 0), stop=(j == len(ks) - 1))
                if s == 0:
                    nc.vector.tensor_copy(macc[side][m], ps)
                else:
                    nc.vector.tensor_add(macc[side][m], macc[side][m], ps)

    if stop_phase <= 1:
        _early_out()
        return

    # ---------------- messages ----------------
    def allreduce(src_sb, tag):
        cin = dram.tile([D, G], F32, tag="cc_in", bufs=2, name=f"ccin_{tag}")
        cout = dram.tile([D, G], F32, tag="cc_out", bufs=2, name=f"ccout_{tag}",
                         addr_space="Shared")
        nc.sync.dma_start(out=cin, in_=src_sb)
        nc.gpsimd.collective_compute(
            "AllReduce", mybir.AluOpType.add,
            ins=[cin.opt()], outs=[cout.opt()], replica_groups=rg)
        dst = small.tile([D, G], F32, tag="msgT", bufs=6, name=f"msgT_{tag}")
        nc.sync.dma_start(out=dst, in_=cout)
        return dst

    rawb = {}
    for side in ("u", "i"):
        rawb[side] = small.tile([D, G], BF16, tag=f"rawb_{side}",
                                name=f"rawb_{side}")
        nc.vector.tensor_copy(rawb[side], raw_ps[side])
    mps = psum.tile([D, G], F32, tag="big", name="proj0")
    for (off, w) in gsplit:
        nc.tensor.matmul(mps[:, off:off + w], lhsT=proj_sb[0][0],
                         rhs=rawb["u"][:, off:off + w], start=True, stop=False)
        nc.tensor.matmul(mps[:, off:off + w], lhsT=proj_sb[0][1],
                         rhs=rawb["i"][:, off:off + w], start=False, stop=True)
    m0p = small.tile([D, G], F32, tag="msgT", bufs=6, name="msg0_part")
    nc.vector.tensor_copy(m0p, mps)
    msgT = [None] * L
    msgT[0] = allreduce(m0p, "l0")
    nc.vector.tensor_scalar(out=msgT[0], in0=msgT[0], scalar1=projb_sb[0],
                            scalar2=None, op0=mybir.AluOpType.add)

    if stop_phase <= 2:
        _early_out()
        return

    def transpose_to_nat(srcT_b, tag):
        """[D, G] bf16 -> GK natural tiles [PT, D] bf16 (PE transpose)."""
        nats = []
        for k in range(GK):
            pst = psum.tile([PT, D], BF16, tag="big", name=f"tp_{tag}_{k}")
            nc.tensor.transpose(pst, srcT_b[:, k * PT:(k + 1) * PT], ident_b)
            nt = natp.tile([PT, D], BF16, tag=f"nat_{tag}_{k}",
                           name=f"nat_{tag}_{k}")
            nc.vector.tensor_copy(nt, pst)
            nats.append(nt)
        return nats

    for l in range(1, L):
        mb = small.tile([D, G], BF16, tag="msgTb", name=f"msgTb_{l}")
        nc.vector.tensor_copy(mb, msgT[l - 1])
        zps = psum.tile([D, G], F32, tag="big", name=f"z_{l}")
        for si, side in enumerate(("u", "i")):
            qps = psum.tile([D, G], F32, tag="big", name=f"q_{l}_{side}")
            for (off, w) in gsplit:
                nc.tensor.matmul(qps[:, off:off + w], lhsT=proj_sb[l][si],
                                 rhs=mb[:, off:off + w], start=True, stop=True)
            qb = small.tile([D, G], BF16, tag="qTb", name=f"qTb_{l}_{side}")
            nc.vector.tensor_copy(qb, qps)
            qn = transpose_to_nat(qb, f"q{l}{side}")
            for k in range(GK):
                for (off, w) in gsplit:
                    nc.tensor.matmul(
                        zps[:, off:off + w], lhsT=qn[k],
                        rhs=macc[side][k][:, off:off + w],
                        start=(si == 0 and k == 0),
                        stop=(si == 1 and k == GK - 1))
        zp = small.tile([D, G], F32, tag="msgT", bufs=6, name=f"z_part_{l}")
        nc.vector.tensor_copy(zp, zps)
        msgT[l] = allreduce(zp, f"l{l}")
        nc.vector.tensor_scalar(out=msgT[l], in0=msgT[l], scalar1=projb_sb[l],
                                scalar2=None, op0=mybir.AluOpType.add)

    msum = small.tile([D, G], F32, tag="msgT", bufs=6, name="msg_sum_T")
    nc.vector.tensor_add(msum, msgT[0], msgT[1])
    for l in range(2, L):
        nc.vector.tensor_add(msum, msum, msgT[l])
    msn = []
    for k in range(GK):
        pst = psum.tile([PT, D], F32, tag="big", name=f"tp_ms_{k}")
        nc.tensor.transpose(pst, msum[:, k * PT:(k + 1) * PT], ident_f)
        nt = natp.tile([PT, D], F32R, tag=f"nat_ms_{k}", name=f"nat_ms_{k}")
        nc.vector.tensor_copy(nt, pst)
        msn.append(nt)

    if stop_phase <= 3:
        _early_out()
        return

    # ---------------- final item embeddings ----------------
    iemb_b = dram.tile([IC, D], F32, tag="iemb_local")
    iemb_all = dram.tile([NC * IC, D], F32, tag="iemb_all",
                         addr_space="Shared")
    ecsplit = _nsplit(EC)
    for c in range(IC // EC):
        eps = psum.tile([D, EC], F32, tag="big", name=f"embT_{c}")
        for k in range(GK):
            ft = streams.tile([PT, EC], F32R,
                              tag=("hstream" if c % 2 == 0 else "fstream"),
                              name=f"fiT_{c}_{k}")
            nc.sync.dma_start(
                out=ft, in_=io["fi_t"][k * PT:(k + 1) * PT, c * EC:(c + 1) * EC])
            for (off, w) in ecsplit:
                nc.tensor.matmul(eps[:, off:off + w], lhsT=msn[k],
                                 rhs=ft[:, off:off + w],
                                 start=(k == 0), stop=(k == GK - 1))
        ets = small.tile([D, EC], F32, tag="embT_sb", name=f"embT_sb_{c}")
        nc.vector.tensor_copy(ets, eps)
        for t in range(EC // PT):
            kg = c * (EC // PT) + t
            pst = psum.tile([PT, D], F32, tag="big", name=f"etp_{c}_{t}")
            nc.tensor.transpose(pst, ets[:, t * PT:(t + 1) * PT], ident_f)
            ie = small.tile([PT, D], F32, tag="iemb_t")
            nc.vector.tensor_add(ie, pst, it0f[:, kg * D:(kg + 1) * D])
            nc.sync.dma_start(out=iemb_b[kg * PT:(kg + 1) * PT, :], in_=ie)

    streams.release()
    gath = tc.alloc_tile_pool(name="gath", bufs=2)

    nc.gpsimd.collective_compute(
        "AllGather", mybir.AluOpType.bypass,
        ins=[iemb_b.opt()], outs=[iemb_all.opt()], replica_groups=rg)

    if stop_phase <= 4:
        _early_out()
        return

    # ---------------- attention / BPR head (batch shard) ----------------
    pred_in = dram.tile([2, BC], F32, tag="pred_in")
    pred_ag = dram.tile([2 * NC, BC], F32, tag="pred_ag",
                        addr_space="Shared")

    boff = 0
    while boff < BC:
        bp = min(128, BC - boff)
        idx = gath.tile([bp, H], I32, tag="idx")
        nc.sync.dma_start(out=idx, in_=io["hist_idx"][boff:boff + bp, :])
        pidx = gath.tile([bp, 1], I32, tag="pidx")
        nc.sync.dma_start(out=pidx, in_=io["pos_idx"][boff:boff + bp, :])
        nidx = gath.tile([bp, 1], I32, tag="nidx")
        nc.sync.dma_start(out=nidx, in_=io["neg_idx"][boff:boff + bp, :])
        mski = gath.tile([bp, H], I32, tag="mski")
        nc.sync.dma_start(out=mski, in_=io["mask_i"][boff:boff + bp, :])

        hist = gath.tile([bp, H * D], F32, tag="hist")
        # HW indirect DMA consumes ONE index per partition -> one gather per h
        for h in range(H):
            nc.gpsimd.indirect_dma_start(
                out=hist[:, h * D:(h + 1) * D], out_offset=None, in_=iemb_all,
                in_offset=bass.IndirectOffsetOnAxis(ap=idx[:, h:h + 1], axis=0))
        ipos = gath.tile([bp, D], F32, tag="ipos")
        nc.gpsimd.indirect_dma_start(
            out=ipos, out_offset=None, in_=iemb_all,
            in_offset=bass.IndirectOffsetOnAxis(ap=pidx[:, :], axis=0))
        ineg = gath.tile([bp, D], F32, tag="ineg")
        nc.gpsimd.indirect_dma_start(
            out=ineg, out_offset=None, in_=iemb_all,
            in_offset=bass.IndirectOffsetOnAxis(ap=nidx[:, :], axis=0))

        maskf = gath.tile([bp, H], F32, tag="maskf")
        nc.vector.tensor_copy(maskf, mski)

        # logits[b, h] = sum_d hist[b, h, d] * attn_w[d]
        # (tensor_tensor_reduce is broken on this HW path -> NRT 101; use
        # scalar_tensor_tensor with accum_out instead)
        logits = gath.tile([bp, H], F32, tag="logits")
        lsc = gath.tile([bp, D], F32, tag="lsc")
        for h in range(H):
            nc.vector.scalar_tensor_tensor(
                out=lsc, in0=hist[:, h * D:(h + 1) * D], scalar=1.0,
                in1=attn_sb[:bp, :], op0=mybir.AluOpType.mult,
                op1=mybir.AluOpType.mult, accum_out=logits[:, h:h + 1])

        # masked softmax over H (mask entries are exactly 0/1)
        neg_big = gath.tile([bp, H], F32, tag="neg_big")
        nc.vector.tensor_scalar(out=neg_big, in0=maskf, scalar1=1.0,
                                scalar2=60.0, op0=mybir.AluOpType.subtract,
                                op1=mybir.AluOpType.mult)
        ml = gath.tile([bp, H], F32, tag="ml")
        nc.vector.tensor_mul(ml, logits, maskf)
        nc.vector.tensor_add(ml, ml, neg_big)
        rmax = gath.tile([bp, 1], F32, tag="rmax")
        nc.vector.reduce_max(out=rmax, in_=ml, axis=mybir.AxisListType.X)
        nc.vector.tensor_scalar(out=ml, in0=ml, scalar1=rmax, scalar2=None,
                                op0=mybir.AluOpType.subtract)
        ex = gath.tile([bp, H], F32, tag="ex")
        nc.scalar.activation(ex, ml, mybir.ActivationFunctionType.Exp)
        nc.vector.tensor_mul(ex, ex, maskf)
        ssum = gath.tile([bp, 1], F32, tag="ssum")
        nc.vector.reduce_sum(out=ssum, in_=ex, axis=mybir.AxisListType.X)
        rinv = gath.tile([bp, 1], F32, tag="rinv")
        nc.vector.reciprocal(rinv, ssum)
        nc.vector.tensor_scalar(out=ex, in0=ex, scalar1=rinv, scalar2=None,
                                op0=mybir.AluOpType.mult)

        # g = sum_h attn[b, h] * hist[b, h, :]
        g = gath.tile([bp, D], F32, tag="g")
        nc.vector.tensor_scalar(out=g, in0=hist[:, 0:D], scalar1=ex[:, 0:1],
                                scalar2=None, op0=mybir.AluOpType.mult)
        for h in range(1, H):
            nc.vector.scalar_tensor_tensor(
                out=g, in0=hist[:, h * D:(h + 1) * D], scalar=ex[:, h:h + 1],
                in1=g, op0=mybir.AluOpType.mult, op1=mybir.AluOpType.add)

        pp = gath.tile([bp, D], F32, tag="pp")
        posp = gath.tile([bp, 1], F32, tag="posp")
        nc.vector.tensor_mul(pp, g, ipos)
        nc.vector.reduce_sum(out=posp, in_=pp, axis=mybir.AxisListType.X)
        negp = gath.tile([bp, 1], F32, tag="negp")
        nc.vector.tensor_mul(pp, g, ineg)
        nc.vector.reduce_sum(out=negp, in_=pp, axis=mybir.AxisListType.X)

        nc.sync.dma_start(out=pred_in[0, boff:boff + bp], in_=posp)
        nc.sync.dma_start(out=pred_in[1, boff:boff + bp], in_=negp)
        boff += bp

    if stop_phase <= 5:
        _early_out()
        return

    nc.gpsimd.collective_compute(
        "AllGather", mybir.AluOpType.bypass,
        ins=[pred_in.opt()], outs=[pred_ag.opt()], replica_groups=rg)

    # ---------------- loss (computed redundantly on every core) --------------
    ag3 = pred_ag[:].rearrange("(n two) b -> n two b", two=2)
    pall = gath.tile([NC, BC], F32, tag="pall")
    nc.sync.dma_start(out=pall, in_=ag3[:, 0, :])
    nall = gath.tile([NC, BC], F32, tag="nall")
    nc.sync.dma_start(out=nall, in_=ag3[:, 1, :])
    x = gath.tile([NC, BC], F32, tag="x")
    nc.vector.tensor_sub(x, nall, pall)
    # softplus(x) for small |x| (preds are O(0.1)): even-poly Taylor series,
    # abs err < 2e-6 for |x| <= 1.  (No Softplus/Ln ACT table on TRN2.)
    y = gath.tile([NC, BC], F32, tag="y")
    nc.vector.tensor_mul(y, x, x)
    sp = gath.tile([NC, BC], F32, tag="sp")
    nc.vector.tensor_scalar(out=sp, in0=y, scalar1=1.0 / 2880.0,
                            scalar2=-1.0 / 192.0, op0=mybir.AluOpType.mult,
                            op1=mybir.AluOpType.add)
    nc.vector.tensor_mul(sp, sp, y)
    nc.vector.tensor_scalar(out=sp, in0=sp, scalar1=0.125, scalar2=None,
                            op0=mybir.AluOpType.add)
    nc.vector.tensor_mul(sp, sp, y)
    xh = gath.tile([NC, BC], F32, tag="xh")
    nc.vector.tensor_scalar(out=xh, in0=x, scalar1=0.5,
                            scalar2=float(np.log(2.0)),
                            op0=mybir.AluOpType.mult, op1=mybir.AluOpType.add)
    nc.vector.tensor_add(sp, sp, xh)
    prt = gath.tile([NC, 1], F32, tag="prt")
    nc.vector.reduce_sum(out=prt, in_=sp, axis=mybir.AxisListType.X)
    lps = psum.tile([1, 1], F32, tag="big", name="loss_ps")
    nc.tensor.matmul(lps, lhsT=prt, rhs=ones8, start=True, stop=True)
    lsb = gath.tile([1, 1], F32, tag="lsb")
    nc.scalar.activation(lsb, lps, mybir.ActivationFunctionType.Copy,
                         scale=1.0 / B)
    nc.sync.dma_start(out=io["loss_out"][0:1], in_=lsb)
    nc.sync.dma_start(
        out=io["pos_out"].rearrange("(n b) -> n b", b=BC), in_=ag3[:, 0, :])

    for p in (gath, dram, psum, natp, small, macc_p, const):
        p.release()


# ---------------------------------------------------------------------------
# host side
# ---------------------------------------------------------------------------

def shard_inputs(cfg: Cfg, inputs: dict) -> list:
    U, I, G, D, L, B, H, NC = (cfg.U, cfg.I, cfg.G, cfg.D, cfg.L, cfg.B,
                               cfg.H, cfg.NC)
    UC, IC, BC = cfg.UC, cfg.IC, cfg.BC
    f32 = np.float32
    Hu = np.asarray(inputs["user_hyper"], f32)
    Hi = np.asarray(inputs["item_hyper"], f32)
    F = np.asarray(inputs["full_hyper"], f32)
    user_w = np.asarray(inputs["user_w"], f32)
    item_w = np.asarray(inputs["item_w"], f32)
    agg_w = np.asarray(inputs["agg_w"], f32)
    agg_b = np.asarray(inputs["agg_b"], f32)
    attn_w = np.asarray(inputs["attn_w"], f32)
    hist = np.asarray(inputs["group_history"]).astype(np.int32)
    mask = np.asarray(inputs["group_mask"]).astype(np.int32)
    pos = np.asarray(inputs["pos_item_inputs"]).astype(np.int32).reshape(B, 1)
    neg = np.asarray(inputs["neg_item_inputs"]).astype(np.int32).reshape(B, 1)

    proj_w = np.stack([
        np.stack([agg_w[l][:, :D].T, agg_w[l][:, D:].T]) for l in range(L)
    ]).astype(f32)
    attn_wt = np.tile(attn_w.reshape(1, D), (128, 1)).astype(f32)

    maps = []
    for k in range(NC):
        us = slice(k * UC, (k + 1) * UC)
        isl = slice(k * IC, (k + 1) * IC)
        bs = slice(k * BC, (k + 1) * BC)
        fi_k = F[U:][isl]
        maps.append({
            "hu_t": np.ascontiguousarray(Hu[:, us].T),
            "hi_t": np.ascontiguousarray(Hi[:, isl].T),
            "fu": np.ascontiguousarray(F[:U][us]),
            "fi": np.ascontiguousarray(fi_k),
            "fi_t": np.ascontiguousarray(fi_k.T),
            "u0": np.ascontiguousarray(user_w[us]),
            "it0": np.ascontiguousarray(item_w[isl]),
            "proj_w": proj_w,
            "proj_b": np.ascontiguousarray(agg_b),
            "attn_wt": attn_wt,
            "hist_idx": np.ascontiguousarray(hist[bs]),
            "pos_idx": np.ascontiguousarray(pos[bs]),
            "neg_idx": np.ascontiguousarray(neg[bs]),
            "mask_i": np.ascontiguousarray(mask[bs]),
        })
    return maps


_CACHE = {}


def get_nc(cfg: Cfg, debug=False):
    key = (tuple(sorted((k, v) for k, v in cfg.__dict__.items())), debug)
    if key not in _CACHE:
        _CACHE[key] = build_nc(cfg, debug=debug)
    return _CACHE[key]


def kernel(**inputs):
    cfg = Cfg()
    nc = get_nc(cfg)
    in_maps = shard_inputs(cfg, inputs)
    res = bass_utils.run_bass_kernel_spmd(
        nc, in_maps, core_ids=list(range(cfg.NC)))
    out = res.results[0]
    loss = np.float32(np.asarray(out["loss_out"]).reshape(())[()])
    pos_pred = np.asarray(out["pos_out"], np.float32).reshape(cfg.B)
    return loss, pos_pred


# revision 21
# speedup vs baseline: 1.4191x; 1.3365x over previous
"""Trainium2 Bass kernel for nn_AlignGroup (hypergraph GNN message passing).

Algorithm (algebraically equivalent to the reference):
  Only i_emb = item_w + F_i @ (msg_0 + msg_1 + msg_2) is needed for the
  outputs, where msg_l are the [G, D] hyperedge messages.  Layers 1 and 2
  are collapsed through the [G, G] operators M_u = Hu @ F_u, M_i = Hi @ F_i:
      msg_0 = (Hu @ u0) @ A1_0 + (Hi @ it0) @ A2_0 + b_0
      msg_l = M_u @ (msg_{l-1} @ A1_l) + M_i @ (msg_{l-1} @ A2_l) + b_l
  so every big matrix is streamed from HBM exactly once (memory roofline)
  instead of once per layer.

Sharding (8 cores): contraction-dim sharding of the node axis (U and I each
split 8 ways).  Each core holds column-shards of Hu/Hi (pre-transposed on
host), row-shards of F, and computes partial M_u^T/M_i^T (kept core-local in
SBUF) plus partial raw messages.  Only [64, G] message partials are
all-reduced (3x 256KB).  The final item embeddings are computed per-shard,
all-gathered, and the attention/BPR head runs data-parallel over the batch;
every core ends with the full outputs (host just reads core 0).

All DMA tiles use 128 partitions (125-partition tiles measured at 1/3 of
HBM line rate); node and group dims are zero-padded to 128 multiples on the
host -- the padding provably contributes exact zeros through every matmul.
Big matmuls run in float32r (full-rate fp32); small glue in bf16/fp32.
"""

import numpy as np

import concourse.bass as bass
import concourse.mybir as mybir
import concourse.tile as tile
from concourse import bacc
from concourse import bass_utils
from concourse.masks import make_identity

F32 = mybir.dt.float32
F32R = mybir.dt.float32r
BF16 = mybir.dt.bfloat16
I32 = mybir.dt.int32

PT = 128  # partition tile (always 128: DMA needs full partitions for BW)


def _pad(n):
    return (n + PT - 1) // PT * PT


class Cfg:
    """Problem/tiling configuration. Defaults = the real problem."""

    def __init__(self, U=20000, I=40000, G=1000, D=64, L=3, B=2048, H=50,
                 NC=8, SUP=5):
        self.U, self.I, self.G, self.D, self.L, self.B, self.H = U, I, G, D, L, B, H
        self.NC = NC
        self.UC, self.IC, self.BC = U // NC, I // NC, B // NC
        self.UCp, self.ICp, self.Gp = _pad(self.UC), _pad(self.IC), _pad(G)
        self.UK, self.IK = self.UCp // PT, self.ICp // PT
        self.GK = self.Gp // PT
        self.SUP = SUP            # k-tiles per streaming super-tile
        self.EC = min(1024, self.ICp)   # emb column chunk
        assert self.ICp % self.EC == 0
        assert self.Gp * 4 <= 4096  # [*, Gp] fp32 psum tile must fit 2 banks


def _nsplit(n, cap=512):
    out, off = [], 0
    while off < n:
        w = min(cap, n - off)
        out.append((off, w))
        off += w
    return out


def build_nc(cfg: Cfg, debug: bool = False, stop_phase: int = 99) -> bacc.Bacc:
    nc = bacc.Bacc("TRN2", target_bir_lowering=False, debug=debug,
                   num_devices=cfg.NC)
    D, L, B, H = cfg.D, cfg.L, cfg.B, cfg.H
    UCp, ICp, Gp, BC = cfg.UCp, cfg.ICp, cfg.Gp, cfg.BC

    io = {}
    def din(name, shape, dtype=F32):
        io[name] = nc.dram_tensor(name, shape, dtype, kind="ExternalInput").ap()
    din("hu_t", [UCp, Gp], F32R); din("hi_t", [ICp, Gp], F32R)
    din("fu", [UCp, Gp], F32R); din("fi", [ICp, Gp], F32R)
    din("fi_t", [Gp, ICp], F32R)
    din("u0", [UCp, D], F32R); din("it0", [ICp, D], F32R)
    din("proj_w", [L, 2, D, D]); din("proj_b", [L, D]); din("attn_wt", [128, D])
    din("hist_idx", [BC, H], I32); din("pos_idx", [BC, 1], I32)
    din("neg_idx", [BC, 1], I32); din("mask_i", [BC, H], I32)
    io["loss_out"] = nc.dram_tensor("loss_out", [1], F32, kind="ExternalOutput").ap()
    io["pos_out"] = nc.dram_tensor("pos_out", [B], F32, kind="ExternalOutput").ap()

    with tile.TileContext(nc) as tc:
        _emit(tc, cfg, io, stop_phase)
    nc.compile()
    return nc


def _emit(tc, cfg, io, stop_phase=99):
    nc = tc.nc
    U, I, G, D, L, B, H = cfg.U, cfg.I, cfg.G, cfg.D, cfg.L, cfg.B, cfg.H
    NC, UC, IC, BC = cfg.NC, cfg.UC, cfg.IC, cfg.BC
    UCp, ICp, Gp = cfg.UCp, cfg.ICp, cfg.Gp
    UK, IK, GK, SUP, EC = cfg.UK, cfg.IK, cfg.GK, cfg.SUP, cfg.EC
    rg = [list(range(NC))]
    gsplit = _nsplit(Gp)
    ecsplit = _nsplit(EC)

    # ---------------- pools ----------------
    # `streams` is allocated last (stack top) and released after the final
    # embedding phase; the gather-phase pool then reuses its SBUF region.
    const = tc.alloc_tile_pool(name="const", bufs=1)
    macc_p = tc.alloc_tile_pool(name="macc", bufs=1)
    small = tc.alloc_tile_pool(name="small", bufs=2)
    natp = tc.alloc_tile_pool(name="natp", bufs=1)
    psum = tc.alloc_tile_pool(name="psum", bufs=4, space="PSUM")
    dram = tc.alloc_tile_pool(name="dram", bufs=1, space="DRAM")
    streams = tc.alloc_tile_pool(name="streams", bufs=2 * SUP)
    gath = None

    # ---------------- constants ----------------
    ident_b = const.tile([D, D], BF16, tag="ident_b")
    make_identity(nc, ident_b)
    ident_f = const.tile([D, D], F32, tag="ident_f")
    make_identity(nc, ident_f)

    proj_sb, projb_sb = [], []
    for l in range(L):
        row = []
        for s in range(2):
            t = const.tile([D, D], BF16, tag=f"proj_{l}_{s}")
            nc.gpsimd.dma_start(out=t, in_=io["proj_w"][l, s])
            row.append(t)
        proj_sb.append(row)
        bt = const.tile([D, 1], F32, tag=f"projb_{l}")
        nc.sync.dma_start(out=bt, in_=io["proj_b"][l, :])
        projb_sb.append(bt)

    attn_sb = const.tile([128, D], F32, tag="attn")
    nc.sync.dma_start(out=attn_sb, in_=io["attn_wt"])

    # layer-0 node features, packed [PT, K*D] (k-major in free dim)
    u0b = const.tile([PT, UK * D], F32R, tag="u0b")
    nc.sync.dma_start(
        out=u0b[:].rearrange("p (k d) -> p k d", k=UK),
        in_=io["u0"].rearrange("(k p) d -> p k d", p=PT))
    it0b = const.tile([PT, IK * D], F32R, tag="it0b")
    nc.sync.dma_start(
        out=it0b[:].rearrange("p (k d) -> p k d", k=IK),
        in_=io["it0"].rearrange("(k p) d -> p k d", p=PT))
    it0f = it0b

    ones8 = const.tile([NC, 1], F32, tag="ones8")
    nc.vector.memset(ones8, 1.0)

    # tiny throwaway AllReduce up front: warms the ncfw collective path
    # while pass-1 streams, so the first real AR doesn't pay cold-start
    wrm_i = dram.tile([NC, 16], F32, tag="wrm_i", name="wrm_i")
    wrm_o = dram.tile([NC, 16], F32, tag="wrm_o", name="wrm_o",
                      addr_space="Shared")
    wrm_s = const.tile([NC, 16], F32, tag="wrm_s")
    nc.vector.memset(wrm_s, 0.0)
    nc.sync.dma_start(out=wrm_i, in_=wrm_s)
    nc.gpsimd.collective_compute(
        "AllReduce", mybir.AluOpType.add,
        ins=[wrm_i.opt()], outs=[wrm_o.opt()], replica_groups=rg)

    def _early_out():
        g = gath if gath is not None else small
        zt = g.tile([NC, BC], F32, tag="pall", name="zero_out")
        nc.vector.memset(zt, 0.0)
        nc.sync.dma_start(
            out=io["pos_out"].rearrange("(n b) -> n b", b=BC), in_=zt)
        z1 = g.tile([1, 1], F32, tag="lsb", name="zero_loss")
        nc.vector.memset(z1, 0.0)
        nc.sync.dma_start(out=io["loss_out"][0:1], in_=z1)
        pools = [gath] if gath is not None else [streams]
        for p in pools + [dram, psum, natp, small, macc_p, const]:
            p.release()

    # ---------------- pass 1: stream Hu^T/Fu then Hi^T/Fi ----------------
    macc, raw_ps = {}, {}
    sides = (("u", io["hu_t"], io["fu"], UK, u0b),
             ("i", io["hi_t"], io["fi"], IK, it0b))
    for side, h_in, f_in, KT, w0sb in sides:
        macc[side] = [macc_p.tile([PT, Gp], BF16, tag=f"macc_{side}_{m}",
                                  name=f"macc_{side}_{m}")
                      for m in range(GK)]
        raw_ps[side] = psum.tile([D, Gp], F32, tag="big", name=f"raw_{side}")
        n_sup = (KT + SUP - 1) // SUP
        for s in range(n_sup):
            ks = list(range(s * SUP, min((s + 1) * SUP, KT)))
            hT, fT = {}, {}
            for k in ks:
                hT[k] = streams.tile([PT, Gp], F32R, tag="hstream",
                                     name=f"h_{side}_{k}")
                nc.sync.dma_start(out=hT[k], in_=h_in[k * PT:(k + 1) * PT, :])
                fT[k] = streams.tile([PT, Gp], F32R, tag="fstream",
                                     name=f"f_{side}_{k}")
                nc.sync.dma_start(out=fT[k], in_=f_in[k * PT:(k + 1) * PT, :])
            # raw0 partial: [D, Gp] += w0[k].T @ hT[k]
            for k in ks:
                for (off, w) in gsplit:
                    nc.tensor.matmul(
                        raw_ps[side][:, off:off + w],
                        lhsT=w0sb[:, k * D:(k + 1) * D],
                        rhs=hT[k][:, off:off + w],
                        start=(k == 0), stop=(k == KT - 1))
            # M^T partial: [Gp(m), Gp] += f[k][:, m].T @ hT[k]
            for m in range(GK):
                ps = psum.tile([PT, Gp], F32, tag="big", name=f"mps_{side}_{s}_{m}")
                for j, k in enumerate(ks):
                    for (off, w) in gsplit:
                        nc.tensor.matmul(
                            ps[:, off:off + w],
                            lhsT=fT[k][:, m * PT:(m + 1) * PT],
                            rhs=hT[k][:, off:off + w],
                            start=(j == 0), stop=(j == len(ks) - 1))
                if s == 0:
                    nc.vector.tensor_copy(macc[side][m], ps)
                else:
                    nc.vector.tensor_add(macc[side][m], macc[side][m], ps)

    if stop_phase <= 1:
        _early_out()
        return

    # ---------------- messages ----------------
    def allreduce(src_sb, tag):
        cin = dram.tile([D, Gp], F32, tag="cc_in", bufs=2, name=f"ccin_{tag}")
        cout = dram.tile([D, Gp], F32, tag="cc_out", bufs=2, name=f"ccout_{tag}",
                         addr_space="Shared")
        nc.sync.dma_start(out=cin, in_=src_sb)
        nc.gpsimd.collective_compute(
            "AllReduce", mybir.AluOpType.add,
            ins=[cin.opt()], outs=[cout.opt()], replica_groups=rg)
        dst = small.tile([D, Gp], F32, tag="msgT", bufs=6, name=f"msgT_{tag}")
        nc.sync.dma_start(out=dst, in_=cout)
        return dst

    rawb = {}
    for side in ("u", "i"):
        rawb[side] = small.tile([D, Gp], BF16, tag=f"rawb_{side}", bufs=1,
                                name=f"rawb_{side}")
        nc.vector.tensor_copy(rawb[side], raw_ps[side])
    mps = psum.tile([D, Gp], F32, tag="big", name="proj0")
    for (off, w) in gsplit:
        nc.tensor.matmul(mps[:, off:off + w], lhsT=proj_sb[0][0],
                         rhs=rawb["u"][:, off:off + w], start=True, stop=False)
        nc.tensor.matmul(mps[:, off:off + w], lhsT=proj_sb[0][1],
                         rhs=rawb["i"][:, off:off + w], start=False, stop=True)
    m0p = small.tile([D, Gp], F32, tag="msgT", bufs=6, name="msg0_part")
    nc.vector.tensor_copy(m0p, mps)
    msgT = [None] * L
    msgT[0] = allreduce(m0p, "l0")
    nc.vector.tensor_scalar(out=msgT[0], in0=msgT[0], scalar1=projb_sb[0],
                            scalar2=None, op0=mybir.AluOpType.add)

    if stop_phase <= 2:
        _early_out()
        return

    def transpose_to_nat(srcT_b, tag):
        """[D, Gp] bf16 -> GK natural tiles [PT, D] bf16 (PE transpose)."""
        nats = []
        for k in range(GK):
            pst = psum.tile([PT, D], BF16, tag="big", name=f"tp_{tag}_{k}")
            nc.tensor.transpose(pst, srcT_b[:, k * PT:(k + 1) * PT], ident_b)
            nt = natp.tile([PT, D], BF16, tag=f"nat_{tag}_{k}",
                           name=f"nat_{tag}_{k}")
            nc.vector.tensor_copy(nt, pst)
            nats.append(nt)
        return nats

    for l in range(1, L):
        mb = small.tile([D, Gp], BF16, tag="msgTb", name=f"msgTb_{l}")
        nc.vector.tensor_copy(mb, msgT[l - 1])
        zps = psum.tile([D, Gp], F32, tag="big", name=f"z_{l}")
        for si, side in enumerate(("u", "i")):
            qps = psum.tile([D, Gp], F32, tag="big", name=f"q_{l}_{side}")
            for (off, w) in gsplit:
                nc.tensor.matmul(qps[:, off:off + w], lhsT=proj_sb[l][si],
                                 rhs=mb[:, off:off + w], start=True, stop=True)
            qb = small.tile([D, Gp], BF16, tag="qTb", name=f"qTb_{l}_{side}")
            nc.vector.tensor_copy(qb, qps)
            qn = transpose_to_nat(qb, f"q{l}{side}")
            for k in range(GK):
                for (off, w) in gsplit:
                    nc.tensor.matmul(
                        zps[:, off:off + w], lhsT=qn[k],
                        rhs=macc[side][k][:, off:off + w],
                        start=(si == 0 and k == 0),
                        stop=(si == 1 and k == GK - 1))
        zp = small.tile([D, Gp], F32, tag="msgT", bufs=6, name=f"z_part_{l}")
        nc.vector.tensor_copy(zp, zps)
        msgT[l] = allreduce(zp, f"l{l}")
        nc.vector.tensor_scalar(out=msgT[l], in0=msgT[l], scalar1=projb_sb[l],
                                scalar2=None, op0=mybir.AluOpType.add)

    if stop_phase <= 3:
        _early_out()
        return

    msum = small.tile([D, Gp], F32, tag="msgT", bufs=6, name="msg_sum_T")
    nc.vector.tensor_add(msum, msgT[0], msgT[1])
    for l in range(2, L):
        nc.vector.tensor_add(msum, msum, msgT[l])
    msn = []
    for k in range(GK):
        pst = psum.tile([PT, D], F32, tag="big", name=f"tp_ms_{k}")
        nc.tensor.transpose(pst, msum[:, k * PT:(k + 1) * PT], ident_f)
        nt = natp.tile([PT, D], F32R, tag=f"nat_ms_{k}", name=f"nat_ms_{k}")
        nc.vector.tensor_copy(nt, pst)
        msn.append(nt)

    # ---------------- final item embeddings ----------------
    iemb_b = dram.tile([IC, D], F32, tag="iemb_local")
    iemb_all = dram.tile([NC * IC, D], F32, tag="iemb_all",
                         addr_space="Shared")
    for c in range(ICp // EC):
        eps = psum.tile([D, EC], F32, tag="big", name=f"embT_{c}")
        for k in range(GK):
            # fiT streams reuse the pass-1 stream slots (released by then);
            # alternate tags so chunk c+1 prefetches while c computes, and
            # ride the scalar HWDGE queue to dodge sync-queue ordering
            ft = streams.tile([PT, EC], F32R,
                              tag=("hstream" if c % 2 == 0 else "fstream"),
                              name=f"fiT_{c}_{k}")
            nc.scalar.dma_start(
                out=ft, in_=io["fi_t"][k * PT:(k + 1) * PT, c * EC:(c + 1) * EC])
            for (off, w) in ecsplit:
                nc.tensor.matmul(eps[:, off:off + w], lhsT=msn[k],
                                 rhs=ft[:, off:off + w],
                                 start=(k == 0), stop=(k == GK - 1))
        ets = small.tile([D, EC], F32, tag="embT_sb", name=f"embT_sb_{c}")
        nc.vector.tensor_copy(ets, eps)
        for t in range(EC // PT):
            kg = c * (EC // PT) + t
            lo = kg * PT
            nrow = min(PT, IC - lo)
            if nrow <= 0:
                continue
            pst = psum.tile([PT, D], F32, tag="big", name=f"etp_{c}_{t}")
            nc.tensor.transpose(pst, ets[:, t * PT:(t + 1) * PT], ident_f)
            ie = small.tile([PT, D], F32, tag="iemb_t")
            nc.vector.tensor_add(ie, pst, it0f[:, kg * D:(kg + 1) * D])
            nc.sync.dma_start(out=iemb_b[lo:lo + nrow, :], in_=ie[:nrow, :])

    if stop_phase <= 4:
        _early_out()
        return

    streams.release()
    gath = tc.alloc_tile_pool(name="gath", bufs=2)

    nc.gpsimd.collective_compute(
        "AllGather", mybir.AluOpType.bypass,
        ins=[iemb_b.opt()], outs=[iemb_all.opt()], replica_groups=rg)

    # ---------------- attention / BPR head (batch shard) ----------------
    pred_in = dram.tile([2, BC], F32, tag="pred_in")
    pred_ag = dram.tile([2 * NC, BC], F32, tag="pred_ag", addr_space="Shared")

    boff = 0
    while boff < BC:
        bp = min(128, BC - boff)
        idx = gath.tile([bp, H], I32, tag="idx")
        nc.sync.dma_start(out=idx, in_=io["hist_idx"][boff:boff + bp, :])
        pidx = gath.tile([bp, 1], I32, tag="pidx")
        nc.sync.dma_start(out=pidx, in_=io["pos_idx"][boff:boff + bp, :])
        nidx = gath.tile([bp, 1], I32, tag="nidx")
        nc.sync.dma_start(out=nidx, in_=io["neg_idx"][boff:boff + bp, :])
        mski = gath.tile([bp, H], I32, tag="mski")
        nc.sync.dma_start(out=mski, in_=io["mask_i"][boff:boff + bp, :])

        hist = gath.tile([bp, H * D], F32, tag="hist")
        # HW indirect DMA consumes ONE index per partition -> one gather per h
        for h in range(H):
            nc.gpsimd.indirect_dma_start(
                out=hist[:, h * D:(h + 1) * D], out_offset=None, in_=iemb_all,
                in_offset=bass.IndirectOffsetOnAxis(ap=idx[:, h:h + 1], axis=0))
        ipos = gath.tile([bp, D], F32, tag="ipos")
        nc.gpsimd.indirect_dma_start(
            out=ipos, out_offset=None, in_=iemb_all,
            in_offset=bass.IndirectOffsetOnAxis(ap=pidx[:, :], axis=0))
        ineg = gath.tile([bp, D], F32, tag="ineg")
        nc.gpsimd.indirect_dma_start(
            out=ineg, out_offset=None, in_=iemb_all,
            in_offset=bass.IndirectOffsetOnAxis(ap=nidx[:, :], axis=0))

        maskf = gath.tile([bp, H], F32, tag="maskf")
        nc.vector.tensor_copy(maskf, mski)

        # logits[b, h] = sum_d hist[b, h, d] * attn_w[d]
        # (tensor_tensor_reduce is broken on this HW path -> NRT 101)
        logits = gath.tile([bp, H], F32, tag="logits")
        lsc = gath.tile([bp, D], F32, tag="lsc")
        for h in range(H):
            nc.vector.scalar_tensor_tensor(
                out=lsc, in0=hist[:, h * D:(h + 1) * D], scalar=1.0,
                in1=attn_sb[:bp, :], op0=mybir.AluOpType.mult,
                op1=mybir.AluOpType.mult, accum_out=logits[:, h:h + 1])

        # masked softmax over H (mask entries are exactly 0/1)
        neg_big = gath.tile([bp, H], F32, tag="neg_big")
        nc.vector.tensor_scalar(out=neg_big, in0=maskf, scalar1=1.0,
                                scalar2=60.0, op0=mybir.AluOpType.subtract,
                                op1=mybir.AluOpType.mult)
        ml = gath.tile([bp, H], F32, tag="ml")
        nc.vector.tensor_mul(ml, logits, maskf)
        nc.vector.tensor_add(ml, ml, neg_big)
        rmax = gath.tile([bp, 1], F32, tag="rmax")
        nc.vector.reduce_max(out=rmax, in_=ml, axis=mybir.AxisListType.X)
        nc.vector.tensor_scalar(out=ml, in0=ml, scalar1=rmax, scalar2=None,
                                op0=mybir.AluOpType.subtract)
        ex = gath.tile([bp, H], F32, tag="ex")
        nc.scalar.activation(ex, ml, mybir.ActivationFunctionType.Exp)
        nc.vector.tensor_mul(ex, ex, maskf)
        ssum = gath.tile([bp, 1], F32, tag="ssum")
        nc.vector.reduce_sum(out=ssum, in_=ex, axis=mybir.AxisListType.X)
        rinv = gath.tile([bp, 1], F32, tag="rinv")
        nc.vector.reciprocal(rinv, ssum)
        nc.vector.tensor_scalar(out=ex, in0=ex, scalar1=rinv, scalar2=None,
                                op0=mybir.AluOpType.mult)

        # g = sum_h attn[b, h] * hist[b, h, :]  (two chains halve latency)
        g = gath.tile([bp, D], F32, tag="g")
        g2 = gath.tile([bp, D], F32, tag="g2")
        nc.vector.tensor_scalar(out=g, in0=hist[:, 0:D], scalar1=ex[:, 0:1],
                                scalar2=None, op0=mybir.AluOpType.mult)
        nc.vector.tensor_scalar(out=g2, in0=hist[:, D:2 * D],
                                scalar1=ex[:, 1:2],
                                scalar2=None, op0=mybir.AluOpType.mult)
        for h in range(2, H):
            tgt = g if h % 2 == 0 else g2
            nc.vector.scalar_tensor_tensor(
                out=tgt, in0=hist[:, h * D:(h + 1) * D], scalar=ex[:, h:h + 1],
                in1=tgt, op0=mybir.AluOpType.mult, op1=mybir.AluOpType.add)
        nc.vector.tensor_add(g, g, g2)

        pp = gath.tile([bp, D], F32, tag="pp")
        posp = gath.tile([bp, 1], F32, tag="posp")
        nc.vector.tensor_mul(pp, g, ipos)
        nc.vector.reduce_sum(out=posp, in_=pp, axis=mybir.AxisListType.X)
        negp = gath.tile([bp, 1], F32, tag="negp")
        nc.vector.tensor_mul(pp, g, ineg)
        nc.vector.reduce_sum(out=negp, in_=pp, axis=mybir.AxisListType.X)

        nc.sync.dma_start(out=pred_in[0, boff:boff + bp], in_=posp)
        nc.sync.dma_start(out=pred_in[1, boff:boff + bp], in_=negp)
        boff += bp

    if stop_phase <= 5:
        _early_out()
        return

    nc.gpsimd.collective_compute(
        "AllGather", mybir.AluOpType.bypass,
        ins=[pred_in.opt()], outs=[pred_ag.opt()], replica_groups=rg)

    # ---------------- loss (computed redundantly on every core) --------------
    ag3 = pred_ag[:].rearrange("(n two) b -> n two b", two=2)
    pall = gath.tile([NC, BC], F32, tag="pall")
    nc.sync.dma_start(out=pall, in_=ag3[:, 0, :])
    nall = gath.tile([NC, BC], F32, tag="nall")
    nc.sync.dma_start(out=nall, in_=ag3[:, 1, :])
    x = gath.tile([NC, BC], F32, tag="x")
    nc.vector.tensor_sub(x, nall, pall)
    # softplus(x) for small |x| (preds are O(0.1)): even-poly Taylor series,
    # abs err < 2e-6 for |x| <= 1.  (No Softplus/Ln ACT table on TRN2.)
    y = gath.tile([NC, BC], F32, tag="y")
    nc.vector.tensor_mul(y, x, x)
    sp = gath.tile([NC, BC], F32, tag="sp")
    nc.vector.tensor_scalar(out=sp, in0=y, scalar1=1.0 / 2880.0,
                            scalar2=-1.0 / 192.0, op0=mybir.AluOpType.mult,
                            op1=mybir.AluOpType.add)
    nc.vector.tensor_mul(sp, sp, y)
    nc.vector.tensor_scalar(out=sp, in0=sp, scalar1=0.125, scalar2=None,
                            op0=mybir.AluOpType.add)
    nc.vector.tensor_mul(sp, sp, y)
    xh = gath.tile([NC, BC], F32, tag="xh")
    nc.vector.tensor_scalar(out=xh, in0=x, scalar1=0.5,
                            scalar2=float(np.log(2.0)),
                            op0=mybir.AluOpType.mult, op1=mybir.AluOpType.add)
    nc.vector.tensor_add(sp, sp, xh)
    prt = gath.tile([NC, 1], F32, tag="prt")
    nc.vector.reduce_sum(out=prt, in_=sp, axis=mybir.AxisListType.X)
    lps = psum.tile([1, 1], F32, tag="big", name="loss_ps")
    nc.tensor.matmul(lps, lhsT=prt, rhs=ones8, start=True, stop=True)
    lsb = gath.tile([1, 1], F32, tag="lsb")
    nc.scalar.activation(lsb, lps, mybir.ActivationFunctionType.Copy,
                         scale=1.0 / B)
    nc.sync.dma_start(out=io["loss_out"][0:1], in_=lsb)
    nc.sync.dma_start(
        out=io["pos_out"].rearrange("(n b) -> n b", b=BC), in_=ag3[:, 0, :])

    for p in (gath, dram, psum, natp, small, macc_p, const):
        p.release()


# ---------------------------------------------------------------------------
# host side
# ---------------------------------------------------------------------------

def shard_inputs(cfg: Cfg, inputs: dict) -> list:
    U, I, G, D, L, B, H, NC = (cfg.U, cfg.I, cfg.G, cfg.D, cfg.L, cfg.B,
                               cfg.H, cfg.NC)
    UC, IC, BC = cfg.UC, cfg.IC, cfg.BC
    UCp, ICp, Gp = cfg.UCp, cfg.ICp, cfg.Gp
    f32 = np.float32
    Hu = np.asarray(inputs["user_hyper"], f32)
    Hi = np.asarray(inputs["item_hyper"], f32)
    F = np.asarray(inputs["full_hyper"], f32)
    user_w = np.asarray(inputs["user_w"], f32)
    item_w = np.asarray(inputs["item_w"], f32)
    agg_w = np.asarray(inputs["agg_w"], f32)
    agg_b = np.asarray(inputs["agg_b"], f32)
    attn_w = np.asarray(inputs["attn_w"], f32)
    hist = np.asarray(inputs["group_history"]).astype(np.int32)
    mask = np.asarray(inputs["group_mask"]).astype(np.int32)
    pos = np.asarray(inputs["pos_item_inputs"]).astype(np.int32).reshape(B, 1)
    neg = np.asarray(inputs["neg_item_inputs"]).astype(np.int32).reshape(B, 1)

    proj_w = np.stack([
        np.stack([agg_w[l][:, :D].T, agg_w[l][:, D:].T]) for l in range(L)
    ]).astype(f32)
    attn_wt = np.tile(attn_w.reshape(1, D), (128, 1)).astype(f32)

    def padrc(a, r, c):
        out = np.zeros((r, c), f32)
        out[:a.shape[0], :a.shape[1]] = a
        return out

    maps = []
    for k in range(NC):
        us = slice(k * UC, (k + 1) * UC)
        isl = slice(k * IC, (k + 1) * IC)
        bs = slice(k * BC, (k + 1) * BC)
        fi_k = F[U:][isl]
        maps.append({
            "hu_t": padrc(np.ascontiguousarray(Hu[:, us].T), UCp, Gp),
            "hi_t": padrc(np.ascontiguousarray(Hi[:, isl].T), ICp, Gp),
            "fu": padrc(F[:U][us], UCp, Gp),
            "fi": padrc(fi_k, ICp, Gp),
            "fi_t": padrc(np.ascontiguousarray(fi_k.T), Gp, ICp),
            "u0": padrc(user_w[us], UCp, D),
            "it0": padrc(item_w[isl], ICp, D),
            "proj_w": proj_w,
            "proj_b": np.ascontiguousarray(agg_b),
            "attn_wt": attn_wt,
            "hist_idx": np.ascontiguousarray(hist[bs]),
            "pos_idx": np.ascontiguousarray(pos[bs]),
            "neg_idx": np.ascontiguousarray(neg[bs]),
            "mask_i": np.ascontiguousarray(mask[bs]),
        })
    return maps


_CACHE = {}


def get_nc(cfg: Cfg, debug=False, stop_phase=99):
    key = (tuple(sorted((k, v) for k, v in cfg.__dict__.items())), debug,
           stop_phase)
    if key not in _CACHE:
        _CACHE[key] = build_nc(cfg, debug=debug, stop_phase=stop_phase)
    return _CACHE[key]


def kernel(**inputs):
    cfg = Cfg()
    nc = get_nc(cfg)
    in_maps = shard_inputs(cfg, inputs)
    res = bass_utils.run_bass_kernel_spmd(
        nc, in_maps, core_ids=list(range(cfg.NC)))
    out = res.results[0]
    loss = np.float32(np.asarray(out["loss_out"]).reshape(())[()])
    pos_pred = np.asarray(out["pos_out"], np.float32).reshape(cfg.B)
    return loss, pos_pred
